# revision 22
# baseline (speedup 1.0000x reference)
"""Trainium2 Bass kernel: multi-head causal attention with RoPE (LLaMA-style).

Problem: y = Attention(x) with B=2, S=2048, D=2048, H=16 heads, HD=128,
torch-Linear convention (y = x @ W.T), interleaved-rope, additive mask.

Sharding (8 NeuronCores): batch (2) x head-groups (4) grid.  Core c handles
batch b = c // 4 and heads 4g..4g+3 where g = c % 4 (tensor parallel:
wq/wk/wv column-parallel, wo row-parallel).  Each core returns a partial
y contribution [S, D] (bf16); the host sums the 4 partials per batch.

Layout strategy (no on-chip transposes anywhere):
  - Host pre-packs every input partition-major (one contiguous DRAM run
    per SBUF partition per DMA) so the DMA engines move 4-16KB
    descriptors instead of 1KB ones: xT [P, nSC, nDK, CW],
    w*i [P, nDK, E], woi [P, nH, D].
  - Q^T,K^T computed directly in [hd, s] layout (hd = partitions) with the
    head-dim DEINTERLEAVED (rows 0-63 = even/"re" dims, 64-127 = odd/"im")
    by permuting wq/wk columns on the host; RoPE is then plain 64-partition
    elementwise ops.  The permutation is invisible to Q.K^T contraction.
  - scores are computed TRANSPOSED [sk, sq] so softmax-denominators come
    from a ones-matmul (column sums) and exp(scores)^T feeds the PV matmul
    directly as the moving operand: P^T never materializes.
  - attention out falls out as out^T [hd, sq] = exactly the stationary
    layout the wo row-parallel matmul wants.
Matmul inputs are bf16 (fp32 PSUM accumulation); softmax runs in fp32.

Perf structure (PE matmul pipe is the roofline, ~91% busy):
  - DMA: the two HARDWARE queues (sync ~200GB/s, scalar ~200GB/s) carry
    everything start-critical; the slow gpsimd SOFTWARE queue (~20GB/s)
    carries only maskd + wo (late first use) and mid-kernel y halves.
    First-use order: x chunk0 + wv stream in dk-QUARTERS (0.5MB pieces,
    one ring each) so the V projection starts at ~12us instead of ~23;
    the V dk-loop is split into quarter passes with two s-tiles'
    PSUM banks open so matmuls start on the first quarter.
  - The attention j-loop alone is ACT-bound (exp of a [128,512] tile takes
    ~630ns vs ~430ns of PE work per j), so PE filler matmuls are pumped
    INTO the j-loop: head h's attention carries head h+1's K/Q projection
    matmuls (2 per j), and the last head's attention carries the wo
    output-projection matmuls for m-tiles of already-finished chunks.
  - causal masking of the diagonal [128,128] block is a DVE multiply of
    exp(scores) by the 0/1 maskd tile (off the PE: saves 64 PE matmuls
    vs the old -1e9 mask-add matmul; exp(-1e9)=0 == es*0 exactly).
  - softmax denominators use 4x column-tiling (tile_position=(0,32k), one
    array col-group per j mod 4) so groups of 4 ones-matmuls run
    concurrently on disjoint array column groups (~4x cheaper than a full
    extra pass).  PSUM has_written semantics: start=True clears the bank
    only FOR THE PARTITIONS WRITTEN, so the bank is cleared once per chunk
    by a [P,P]-stationary zero matmul and every ones-matmul runs
    start=False (first write to an element overwrites because its bit is
    clear, later writes accumulate) — validated on HW.
  - ~80 warmup matmuls keep the PE busy from preamble-end so the HAM
    clock-gate reaches 8/8 (2.4 GHz) before real work arrives at ~12us.
  - y goes out as bf16 [128, D] row-blocks alternating sync/gpsimd rings;
    the four FINAL m-tiles stream per-dc on the sync HW ring so the last
    128KB block drains in <1us after its copy.
"""

import math
from contextlib import ExitStack

import numpy as np
import ml_dtypes

P = 128          # partitions / head dim
CW = 512         # s-chunk width (one PSUM bank of fp32)

_built_cache = {}


def _build(*, S, D, E, mask_mode):
    """Build + compile the SPMD Bass program for one core's shard.

    S: sequence length, D: model dim, E: head-columns per core (nH*128).
    mask_mode: 'causal' (use diag block + skip upper triangle),
               'none' (no mask, full attention),
               'generic' (arbitrary additive mask, applied everywhere).
    """
    import concourse.bacc as bacc
    import concourse.mybir as mybir
    import concourse.tile as tile

    f32 = mybir.dt.float32
    bf16 = mybir.dt.bfloat16
    Exp = mybir.ActivationFunctionType.Exp

    nDK = D // P       # k-tiles over model dim
    nH = E // P        # heads on this core
    nSC = S // CW      # 512-wide s-chunks
    nST = S // P       # 128-wide s-tiles
    TPC = CW // P      # s-tiles per chunk (4)
    nDC = D // CW
    QK4 = nDK // 4     # dk-quarter size (V-projection early start)
    HK2 = nDK // 2     # dk-half size (x chunk ring split)
    SCALE = 1.0 / math.sqrt(P)
    causal = mask_mode == "causal"

    nc = bacc.Bacc("TRN2", target_bir_lowering=False, debug=False)

    xT = nc.dram_tensor("xT", [P, nSC, nDK, CW], bf16,
                        kind="ExternalInput").ap()
    wqi = nc.dram_tensor("wqT", [P, nDK, E], bf16, kind="ExternalInput").ap()
    wki = nc.dram_tensor("wkT", [P, nDK, E], bf16, kind="ExternalInput").ap()
    wvi = nc.dram_tensor("wvT", [P, nDK, E], bf16, kind="ExternalInput").ap()
    woi = nc.dram_tensor("woT", [P, nH, D], bf16, kind="ExternalInput").ap()
    cs = nc.dram_tensor("cs", [P, S], bf16, kind="ExternalInput").ap()
    ident = nc.dram_tensor("ident", [P, P], bf16, kind="ExternalInput").ap()
    mneg = nc.dram_tensor("mneg", [P, P], bf16, kind="ExternalInput").ap()
    if mask_mode == "generic":
        maskT = nc.dram_tensor("maskT", [S, S], bf16, kind="ExternalInput").ap()
    y = nc.dram_tensor("y", [S, D], bf16, kind="ExternalOutput").ap()

    with tile.TileContext(nc) as tc, ExitStack() as ctx:
        const = ctx.enter_context(tc.tile_pool(name="const", bufs=1))
        tp = ctx.enter_context(tc.tile_pool(name="tmp", bufs=2))
        expp = ctx.enter_context(tc.tile_pool(name="expp", bufs=8))
        sbB = ctx.enter_context(tc.tile_pool(name="sbB", bufs=2))
        yp = ctx.enter_context(tc.tile_pool(name="yp", bufs=2))
        psS = ctx.enter_context(tc.tile_pool(name="psS", bufs=3, space="PSUM"))
        psO = ctx.enter_context(tc.tile_pool(name="psO", bufs=2, space="PSUM"))
        psD = ctx.enter_context(tc.tile_pool(name="psD", bufs=1, space="PSUM"))
        psP = ctx.enter_context(tc.tile_pool(name="psP", bufs=2, space="PSUM"))

        # ---- persistent tiles --------------------------------------------
        qt = const.tile([P, nH, S], bf16)    # rotated Q^T  (re rows 0-63)
        kt = const.tile([P, nH, S], bf16)    # rotated K^T
        v = const.tile([P, nST, E], bf16)    # V [s within tile, stile, e]
        outT = const.tile([P, nH, S], bf16)  # attention out^T per head
        cs_t = const.tile([P, S], bf16)      # rows 0-63 cos^T, 64-127 sin^T
        id_t = const.tile([P, P], bf16)      # identity (mask-add stationary)
        mn_t = const.tile([P, P], bf16)      # -1e9 strict-lower block
        ones_col = const.tile([P, 1], bf16)
        zmat = const.tile([P, P], bf16)      # zero: psD clears + PE warmup

        nc.vector.memset(ones_col, 1.0)
        nc.vector.memset(zmat, 0.0)

        # PE warmup: keep the tensor engine busy from preamble-end until the
        # first input DMAs land, so the HAM clock-gate is at 8/8 (2.4 GHz)
        # when real work starts and the PE never sees a long idle window.
        wu = psP.tile([1, P], f32, tag="psP", name="wu")
        for _ in range(46):
            nc.tensor.matmul(wu, ones_col, zmat, start=True, stop=True)

        def rope(ps, dst, col):
            """ps: [128, CW] psum raw projection (re rows 0-63, im 64-127).
            dst: [128, CW] bf16 sbuf destination slice. col: s-slice.
            NB the walrus verifier requires all SBUF *inputs* of a
            tensor-tensor op to share a start partition; PSUM inputs and the
            output are exempt, so each product takes one aligned SBUF input
            and the combines read base-0 tiles."""
            re, im = ps[0:64, :], ps[64:128, :]
            cosv, sinv = cs_t[0:64, col], cs_t[64:128, col]
            t1 = tp.tile([64, CW], bf16, tag="t1", name="t1", bufs=1)
            t2 = tp.tile([64, CW], bf16, tag="t2", name="t2", bufs=1)
            nc.vector.tensor_mul(t1, re, cosv)
            nc.vector.tensor_mul(t2, im, sinv)
            nc.vector.tensor_sub(dst[0:64, :], t1, t2)
            t3 = tp.tile([64, CW], bf16, tag="t1", name="t3", bufs=1)
            t4 = tp.tile([64, CW], bf16, tag="t2", name="t4", bufs=1)
            nc.vector.tensor_mul(t3, re, sinv)
            nc.vector.tensor_mul(t4, im, cosv)
            nc.vector.tensor_add(dst[64:128, :], t3, t4)

        # ---- attention chunk emitter -------------------------------------
        # es := exp(scale*scores) runs 3 j-iterations ahead of the PV
        # matmuls; the diagonal block is masked by a DVE multiply with the
        # 0/1 md_t tile (exact: es*0 == exp(score-1e9)).  `filler` is an
        # iterator of callables, each standing for one already-emitted PE
        # matmul of companion work; FPJ are pumped per j so the PE stays
        # ahead of the ACT engine's ~630ns/j exp rate.
        FPJ = 2

        def attn_chunk(h, c, mk=None, filler=None):
            qcol = slice(c * CW, (c + 1) * CW)
            jmax = TPC * c + TPC - 1 if causal else nST - 1
            tiled_den = jmax >= TPC
            # chunk 0 is a serial rope->scores->exp latency chain with only
            # 4 j's of work: pump extra filler per j to keep the PE fed
            boost = 1 if (causal and c == 0 and filler is not None) else 0
            pumped = [0]

            def pump(filler, n):
                if filler is None:
                    return
                for _ in range(n):
                    if next(filler, None) is None:
                        return
                    pumped[0] += 1

            ps_o = psO.tile([P, CW], f32, tag="psO", name="ps_o")
            ps_d = psD.tile([P, CW], f32, tag="psD", name="ps_d")
            ess = {}

            def emit_scores(j):
                o = max(0, j - TPC * c) * P if causal else 0
                ps_s = psS.tile([P, CW], f32, tag="psS", name="ps_s")
                nc.tensor.matmul(
                    ps_s[:, o:], kt[:, h, j * P:(j + 1) * P],
                    qt[:, h, c * CW + o:(c + 1) * CW],
                    start=True, stop=True)
                if causal and c == 0:
                    # chunk 0 sits on the head-boundary critical path: mask
                    # in PSUM via a PE matmul (adds -1e9 strict-lower) so es
                    # needs no post-exp masking op from a busy engine
                    nc.tensor.matmul(ps_s[:, o:o + P], id_t, mn_t,
                                     start=False, stop=True,
                                     skip_group_check=True)
                if causal and c > 0 and j >= TPC * c:
                    # mid-head diag masks: DVE adds the -1e9 block onto the
                    # PSUM scores BEFORE exp.  Keeping the DVE off the es
                    # tiles matters: a DVE reader/writer on a recycled es
                    # buffer makes later exps WAR-wait on the (congested)
                    # DVE queue at every chunk boundary.  (NOT gpsimd:
                    # alternating tensor_tensor with partition_broadcast
                    # forces a ~6us ucode lib swap.)
                    nc.vector.tensor_add(ps_s[:, o:o + P], ps_s[:, o:o + P],
                                         mn_t)
                es = expp.tile([P, CW], bf16, tag="es", name="es", bufs=7)
                nc.scalar.activation(es[:, o:], ps_s[:, o:], Exp, scale=SCALE)
                if mask_mode == "generic":
                    nc.vector.tensor_mul(es, es, mk[:, j, :])
                ess[j] = (es, o)

            for jj in range(min(3, jmax + 1)):
                emit_scores(jj)
                pump(filler, 2 + boost)
            group = []
            for j in range(jmax + 1):
                if j + 3 <= jmax:
                    emit_scores(j + 3)
                es, o = ess[j]
                nc.tensor.matmul(ps_o[:, o:], v[:, j, h * P:(h + 1) * P],
                                 es[:, o:], start=(j == 0), stop=(j == jmax))
                group.append(j)
                if tiled_den and len(group) == 4:
                    # jj<4 runs start=True: clears that partition's whole
                    # bank row (per-partition clear semantics) then writes,
                    # so no separate zero-matmul bank clear is needed; by
                    # j=3 the previous chunk's dd reads have long drained
                    for k, jj in enumerate(group):
                        ees, oo = ess[jj]
                        nc.tensor.matmul(
                            ps_d[32 * k:32 * k + 1, oo:], ones_col,
                            ees[:, oo:], start=(jj < 4),
                            stop=(jj > jmax - 4),
                            skip_group_check=True,
                            tile_position=(0, 32 * k))
                    for jj in group:
                        ess.pop(jj)
                    group = []
                elif not tiled_den and j == jmax:
                    # plain dens, deferred to chunk end for the same reason
                    for jj in range(jmax + 1):
                        ees, oo = ess[jj]
                        nc.tensor.matmul(ps_d[0:1, oo:], ones_col,
                                         ees[:, oo:], start=(jj == 0),
                                         stop=(jj == jmax))
                    for jj in range(jmax + 1):
                        ess.pop(jj)
                if j < jmax:
                    pump(filler, FPJ + boost)
            # Reduce the four denominator rows (tiled) or copy row 0
            # (plain); DVE may read only ONE PSUM input per op, so the rows
            # chain through SBUF temps.  Then normalize out^T by 1/denom.
            dd = tp.tile([1, CW], f32, tag="rr", name="dd", bufs=1)
            if tiled_den:
                da = tp.tile([1, CW], bf16, tag="da", name="da", bufs=1)
                db = tp.tile([1, CW], bf16, tag="db", name="db", bufs=1)
                nc.vector.tensor_copy(da, ps_d[0:1, :])
                nc.vector.tensor_add(db, da, ps_d[32:33, :])
                nc.vector.tensor_add(da, db, ps_d[64:65, :])
                nc.vector.tensor_add(dd, da, ps_d[96:97, :])
            else:
                nc.vector.tensor_copy(dd, ps_d[0:1, :])
            ou = expp.tile([P, CW], bf16, tag="ou", name="ou", bufs=1)
            nc.scalar.copy(ou, ps_o)
            bc = sbB.tile([P, CW], f32, tag="bc", name="bc", bufs=1)
            nc.gpsimd.partition_broadcast(out_ap=bc, in_ap=dd)
            bcr = sbB.tile([P, CW], f32, tag="bcr", name="bcr", bufs=1)
            nc.vector.reciprocal_approx_fast(out=bcr, in_=bc)
            nc.vector.tensor_mul(outT[:, h, qcol], ou, bcr)
            pump(filler, FPJ)
            return pumped[0]

        # ---- output projection (generator: one yield per matmul) ---------
        def wo_mms(m, wo_t, dmae, fast_tail=False, final=False):
            # dc-outer / h-inner: one PSUM bank at a time, LDWEIGHTS per
            # matmul hides under the 512-wide streams.  Mid-kernel m's: one
            # y DMA per half row-block; FINAL m's: per-dc DMAs on the sync
            # HW ring so the last 128KB drains in <1us after its copy.
            yo = yp.tile([P, D], bf16, tag="yo", name="yo")
            half = nDC // 2
            for dc in range(nDC):
                ps = psP.tile([P, CW], f32, tag="psP", name="ps_y")
                for hh in range(nH):
                    nc.tensor.matmul(
                        ps, outT[:, hh, m * P:(m + 1) * P],
                        wo_t[:, hh, dc * CW:(dc + 1) * CW],
                        start=(hh == 0), stop=(hh == nH - 1))
                    yield
                if fast_tail and dc % 2:
                    nc.vector.tensor_copy(yo[:, dc * CW:(dc + 1) * CW], ps)
                else:
                    nc.scalar.copy(yo[:, dc * CW:(dc + 1) * CW], ps)
                if final:
                    fe = nc.sync if dc % 2 == 0 else nc.scalar
                    fe.dma_start(
                        out=y[m * P:(m + 1) * P, dc * CW:(dc + 1) * CW],
                        in_=yo[:, dc * CW:(dc + 1) * CW])
                elif dc == half - 1:
                    nc.sync.dma_start(
                        out=y[m * P:(m + 1) * P, 0:half * CW],
                        in_=yo[:, 0:half * CW])
            if not final:
                nc.gpsimd.dma_start(out=y[m * P:(m + 1) * P, half * CW:],
                                    in_=yo[:, half * CW:])

        # ---- projections + attention, engine-interleaved -----------------
        with tc.tile_pool(name="xw", bufs=1) as xtp, \
             tc.tile_pool(name="wz", bufs=1) as wpool:

            # DMA plan: the two HW rings carry everything start-critical in
            # first-use order.  x chunk0 + wv stream in dk-quarters so the
            # V projection's first dependency is 0.5MB per ring; the other
            # x chunks split dk-halves across both rings; weights follow.
            with tc.tile_pool(name="wv", bufs=1) as wvpool:
                xts = [xtp.tile([P, nDK, CW], bf16, tag=f"xt{sc}", name="xt")
                       for sc in range(nSC)]
                wv_t = wvpool.tile([P, nDK, E], bf16, tag="wv", name="wv")
                for qi in range(4):
                    dks = slice(qi * QK4, (qi + 1) * QK4)
                    nc.sync.dma_start(out=xts[0][:, dks, :],
                                      in_=xT[:, 0, dks, :])
                    nc.scalar.dma_start(out=wv_t[:, dks, :],
                                        in_=wvi[:, dks, :])
                nc.gpsimd.dma_start(out=id_t, in_=ident)
                nc.gpsimd.dma_start(out=mn_t, in_=mneg)
                for sc in range(1, nSC):
                    nc.sync.dma_start(out=xts[sc][:, 0:HK2, :],
                                      in_=xT[:, sc, 0:HK2, :])
                    nc.scalar.dma_start(out=xts[sc][:, HK2:, :],
                                        in_=xT[:, sc, HK2:, :])
                nc.sync.dma_start(out=cs_t, in_=cs)
                wk_t = wpool.tile([P, nDK, E], bf16, tag="wk", name="wk")
                wq_t = wpool.tile([P, nDK, E], bf16, tag="wq", name="wq")
                for kh in range(2):
                    dks = slice(kh * HK2, (kh + 1) * HK2)
                    nc.sync.dma_start(out=wk_t[:, dks, :], in_=wki[:, dks, :])
                    nc.scalar.dma_start(out=wq_t[:, dks, :],
                                        in_=wqi[:, dks, :])

                def xstat(dk, st):
                    xt = xts[st // TPC]
                    o = (st % TPC) * P
                    return xt[:, dk, o:o + P]

                # V projection (all heads at once: rhs = all E columns).
                # Chunk 0's four s-tiles run interleaved over dk-QUARTER
                # passes (4 PSUM banks open: 2 from psP + 2 borrowed from
                # the idle psS pool) so each 3.4us quarter-pass consumes
                # exactly one 0.5MB x-quarter + wv-quarter — matching the
                # early DMA ramp rate with no PE idle.
                pvs = [psP.tile([P, CW], f32, tag="psP", name=f"ps_v{i}")
                       for i in range(2)] + \
                      [psS.tile([P, CW], f32, tag="psS", name=f"ps_v{i}")
                       for i in range(2, 4)]
                for qi in range(4):
                    for sti in range(4):
                        for dk in range(qi * QK4, (qi + 1) * QK4):
                            nc.tensor.matmul(
                                pvs[sti][:, 0:E], xstat(dk, sti),
                                wv_t[:, dk, :],
                                start=(dk == 0), stop=(dk == nDK - 1))
                # V copies run on the DVE: the scalar (ACT) engine's queue
                # holds flow-controlled DMA-issue ops for the weight loads
                # at this point, which would stall the copies ~4us
                for sti in range(4):
                    nc.vector.tensor_copy(v[:, sti, :], pvs[sti][:, 0:E])
                for st in range(4, nST):
                    ps = psP.tile([P, CW], f32, tag="psP", name="ps_v")
                    for dk in range(nDK):
                        nc.tensor.matmul(
                            ps[:, 0:E], xstat(dk, st), wv_t[:, dk, :],
                            start=(dk == 0), stop=(dk == nDK - 1))
                    nc.vector.tensor_copy(v[:, st, :], ps[:, 0:E])

            # wv's SBUF space is free now: wo_t reuses it; the space
            # semaphore naturally delays this DMA past the V projection.
            # (manually entered/exited so pool release stays LIFO wrt xw/wz)
            late_cm = tc.tile_pool(name="late", bufs=1)
            late = late_cm.__enter__()
            if causal:
                wo_t = late.tile([P, nH, D], bf16)
                nc.gpsimd.dma_start(out=wo_t, in_=woi)

            def qk_mms(wts, dest, h, sc):
                esl = slice(h * P, (h + 1) * P)
                col = slice(sc * CW, (sc + 1) * CW)
                ps = psP.tile([P, CW], f32, tag="psP", name="ps_qk")
                for dk in range(nDK):
                    nc.tensor.matmul(
                        ps, wts[:, dk, esl], xts[sc][:, dk, :],
                        start=(dk == 0), stop=(dk == nDK - 1))
                    yield
                rope(ps, dest[:, h, col], col)

            def qk_group(wts, dest, h, sc):
                for _ in qk_mms(wts, dest, h, sc):
                    pass

            def proj_filler(h):
                # one yield per emitted projection matmul of head h; K/Q
                # alternate per chunk so the next head's chunk-0 attention
                # inputs (K0, Q0) are rotated earliest and the head
                # boundary never waits on a rope chain
                for sc in range(nSC):
                    yield from qk_mms(wk_t, kt, h, sc)
                    yield from qk_mms(wq_t, qt, h, sc)

            for sc in range(nSC):
                qk_group(wk_t, kt, 0, sc)
            for sc in range(nSC):
                qk_group(wq_t, qt, 0, sc)

            if causal:
                # Heads 0..nH-2: attention chunks pump head h+1's K/Q
                # projection matmuls (FPJ per j), remainder drained to a
                # per-chunk quota so projections finish with the head.
                # Head nH-1: attention chunks pump wo matmuls of m-tiles
                # from chunks that already finished (lag 1 chunk); the
                # last chunk's wo forms the (PE-dense) tail.
                total_units = 2 * nSC * nDK
                for h in range(nH - 1):
                    stream = proj_filler(h + 1)
                    done = 0
                    for c in range(nSC):
                        done += attn_chunk(h, c, filler=stream)
                        target = (c + 1) * total_units // nSC
                        while done < target:
                            if next(stream, None) is None:
                                break
                            done += 1
                    for _ in stream:
                        pass
                pend = []
                for c in range(nSC):
                    gens = [wo_mms(m, wo_t,
                                   nc.sync if m % 2 == 0 else nc.gpsimd)
                            for m in pend]
                    merged = (u for g in gens for u in g)
                    attn_chunk(nH - 1, c, filler=merged)
                    for _ in merged:
                        pass
                    pend = list(range(TPC * c, TPC * (c + 1)))
                for m in pend:
                    for _ in wo_mms(m, wo_t, nc.sync, fast_tail=True,
                                    final=True):
                        pass
            else:
                for h in range(1, nH):
                    for sc in range(nSC):
                        qk_group(wk_t, kt, h, sc)
                        qk_group(wq_t, qt, h, sc)
                for c in range(nSC):
                    mk = None
                    if mask_mode == "generic":
                        mk = late.tile([P, nST, CW], bf16, tag="mk",
                                       name="mk", bufs=1)
                        nc.sync.dma_start(
                            out=mk,
                            in_=maskT.rearrange("(j p) q -> p j q", p=P)[
                                :, :, c * CW:(c + 1) * CW])
                    for h in range(nH):
                        attn_chunk(h, c, mk=mk)
            if not causal:
                # mk space is released before wo_t is brought in
                late_cm.__exit__(None, None, None)
                late_cm = tc.tile_pool(name="late2", bufs=1)
                late = late_cm.__enter__()
                wo_t = late.tile([P, nH, D], bf16)
                nc.gpsimd.dma_start(out=wo_t, in_=woi)
                for m in range(nST):
                    dmae = nc.sync if m % 2 == 0 else nc.gpsimd
                    for _ in wo_mms(m, wo_t, dmae, final=(m >= nST - 2)):
                        pass
            late_cm.__exit__(None, None, None)

    nc.compile()
    return nc


def _get_built(mask_mode, S, D, E):
    key = (mask_mode, S, D, E)
    if key not in _built_cache:
        _built_cache[key] = _build(S=S, D=D, E=E, mask_mode=mask_mode)
    return _built_cache[key]


def _classify_mask(mask):
    S = mask.shape[0]
    if not mask.any():
        return "none"
    causal = np.where(np.triu(np.ones((S, S), dtype=bool), k=1),
                      np.float32(-1e9), np.float32(0.0))
    if np.array_equal(mask, causal):
        return "causal"
    return "generic"


def make_in_maps(x, wq, wk, wv, wo, freqs_cos, freqs_sin, mask, n_cores=8):
    """Host-side sharding + layout prep. Returns (in_maps, mask_mode, meta).

    Every device input is pre-packed partition-major: for each SBUF
    partition p the bytes it receives are one contiguous DRAM run, so the
    DMA engines move 4-16KB descriptors instead of 1KB ones.
    """
    bf = ml_dtypes.bfloat16
    x = np.asarray(x, np.float32)
    B, S, D = x.shape
    groups = n_cores // B
    E = D // groups
    nH = E // P
    nDK = D // P
    nSC = S // CW

    mask = np.asarray(mask, np.float32)
    mode = _classify_mask(mask)

    fc = np.asarray(freqs_cos, np.float32)
    fs = np.asarray(freqs_sin, np.float32)
    cs = np.concatenate(
        [np.ascontiguousarray(fc.T), np.ascontiguousarray(fs.T)], axis=0
    ).astype(bf)                              # [128, S]
    # masking is applied multiplicatively on exp(scores): exp(mask) — exact
    # 0/1 for the causal -1e9/0 mask
    maskd = np.exp(np.ascontiguousarray(mask[0:P, 0:P].T)).astype(bf)

    # per-head deinterleave: head-local columns [0,2,...,126,1,3,...,127]
    perm1 = np.concatenate([np.arange(0, P, 2), np.arange(1, P, 2)])
    permE = np.concatenate([h * P + perm1 for h in range(nH)])

    wqT_f = np.asarray(wq, np.float32).T      # [D, D]
    wkT_f = np.asarray(wk, np.float32).T
    wvT_f = np.asarray(wv, np.float32).T
    woT_f = np.asarray(wo, np.float32).T      # [E_total, D]

    if mode == "generic":
        maskT_bf = np.exp(np.ascontiguousarray(mask.T)).astype(bf)

    def pack_w(wT):                           # [D, E] -> [P, nDK, E]
        return np.ascontiguousarray(
            wT.reshape(nDK, P, E).transpose(1, 0, 2)).astype(bf)

    # x packed partition-major: [P, nSC, nDK, CW]; the kernel slices
    # dk-quarters/halves of a chunk (contiguous per partition) per DMA
    def pack_x(xb):
        xt = xb.T.reshape(nDK, P, nSC, CW)
        return np.ascontiguousarray(xt.transpose(1, 2, 0, 3)).astype(bf)
    xT_b = [pack_x(x[b]) for b in range(B)]

    in_maps = []
    for c in range(n_cores):
        b, g = divmod(c, groups)
        es = slice(g * E, (g + 1) * E)
        m = {
            "xT": xT_b[b],
            "ident": np.eye(P, dtype=np.float32).astype(bf),
            "mneg": np.where(np.arange(P)[:, None] > np.arange(P)[None, :],
                             np.float32(-1e9), np.float32(0.0)).astype(bf),
            "wqT": pack_w(wqT_f[:, es][:, permE]),
            "wkT": pack_w(wkT_f[:, es][:, permE]),
            "wvT": pack_w(wvT_f[:, es]),
            "woT": np.ascontiguousarray(
                woT_f[es, :].reshape(nH, P, D).transpose(1, 0, 2)).astype(bf),
            "cs": cs,
        }
        if mode == "generic":
            m["maskT"] = maskT_bf
        in_maps.append(m)
    return in_maps, mode, (B, S, D, E, groups)


def kernel(x, wq, wk, wv, wo, freqs_cos, freqs_sin, mask, start_pos=0, **_):
    from concourse.bass_utils import run_bass_kernel_spmd

    in_maps, mode, (B, S, D, E, groups) = make_in_maps(
        x, wq, wk, wv, wo, freqs_cos, freqs_sin, mask)
    nc = _get_built(mode, S, D, E)
    res = run_bass_kernel_spmd(nc, in_maps, core_ids=list(range(len(in_maps))))
    parts = [np.asarray(r["y"], dtype=np.float32) for r in res.results]
    out = np.stack(
        [np.sum(parts[b * groups:(b + 1) * groups], axis=0) for b in range(B)]
    ).astype(np.float32)
    return out
